# revision 2
# baseline (speedup 1.0000x reference)
"""BiGCN (bidirectional 2-layer GCN over many small graphs) on 8 Trainium2 cores.

Strategy: data-parallel over graphs (32 graphs of 128 nodes per core). The
host precomputes each graph's dense *normalized* adjacency
  An = D^{-1/2} (A + I) D^{-1/2}
(transposed, [src, dst] layout) so the device does only dense bf16 matmuls:

  per graph (n=128 nodes), per branch:
    Y   = X @ [W1_td | W1_bu]                  (shared across branches)
    hT  = relu( Y_b^T-contracted: matmul(lhsT=Y_b chunk, rhs=AnT) )  [feat, node]
    Z   = h @ W2h + ones (x) rvec              (rank-1 fold of the root term)
    H2  = relu( matmul(lhsT=AnT, rhs=Z) )      [node, feat]
    readout: mean over nodes via PSUM-accumulated selector matmul,
             root row collected from hT column 0, transposed once at the end.
Final output: concat(TD branch, BU branch) -> [G, 1024].
"""

import numpy as np
import ml_dtypes

import concourse.bass as bass
import concourse.tile as tile
from concourse import bacc, mybir
from concourse.bass_utils import run_bass_kernel_spmd
from concourse.masks import make_identity

# Problem shape (fixed by the task)
N_GRAPHS = 256
N_PER_G = 128
IN_FEATS = 768
H_FEATS = 256
N_CORES = 8
G_PER_CORE = N_GRAPHS // N_CORES            # 32
NODES_PER_CORE = G_PER_CORE * N_PER_G       # 4096
KCH = IN_FEATS // 128                       # 6 feature chunks

MM_DT = mybir.dt.bfloat16
BF16 = ml_dtypes.bfloat16
F32 = mybir.dt.float32
AF = mybir.ActivationFunctionType
OP = mybir.AluOpType


# ----------------------------------------------------------------------------
# Host-side prep: dense normalized adjacency per graph
# ----------------------------------------------------------------------------

def build_adjt(src, dst, n, G):
    """AnT[g, s, d] = norm[d] * A[d, s] * norm[s], A[d, s] = #edges s->d
    (self-loops included in the edge list)."""
    src = np.asarray(src, np.int64)
    dst = np.asarray(dst, np.int64)
    g = dst // n
    if not np.array_equal(src // n, g):
        raise ValueError("cross-graph edge found; contiguous-block sharding invalid")
    A = np.zeros((G, n, n), np.float32)
    np.add.at(A, (g, dst % n, src % n), 1.0)
    deg = A.sum(axis=2)  # in-degree (incl. self-loops); >= 1 by construction
    norm = 1.0 / np.sqrt(deg)
    An = norm[:, :, None] * A * norm[:, None, :]
    return An.transpose(0, 2, 1)  # [G, s, d]


# ----------------------------------------------------------------------------
# Device program (SPMD; one core's shard)
# ----------------------------------------------------------------------------

def build_program(has_bias):
    nc = bacc.Bacc("TRN2", target_bir_lowering=False, debug=False,
                   num_devices=N_CORES)

    def din(name, shape, dt=MM_DT):
        return nc.dram_tensor(name, shape, dt, kind="ExternalInput").ap()

    xt = din("xt", [IN_FEATS, NODES_PER_CORE])
    adjt = din("adjt", [128, G_PER_CORE * 2 * 128])
    xrootst = din("xrootst", [IN_FEATS, G_PER_CORE])
    w1p = din("w1p", [IN_FEATS, 2 * H_FEATS])
    w2h_td = din("w2h_td", [H_FEATS, H_FEATS])
    w2h_bu = din("w2h_bu", [H_FEATS, H_FEATS])
    w2rp = din("w2rp", [IN_FEATS, 2 * H_FEATS])
    meansel = din("meansel", [128, G_PER_CORE * G_PER_CORE])
    if has_bias:
        b1c_td = din("b1c_td", [128, 2], F32)   # b1 as column chunks [feat,1]
        b1c_bu = din("b1c_bu", [128, 2], F32)
        b2b_td = din("b2b_td", [128, H_FEATS], F32)  # b2 broadcast over nodes
        b2b_bu = din("b2b_bu", [128, H_FEATS], F32)
    out = nc.dram_tensor("out", [G_PER_CORE, 4 * H_FEATS], F32,
                         kind="ExternalOutput").ap()

    W2H = {0: w2h_td, 1: w2h_bu}

    with tile.TileContext(nc) as tc:
        with (
            tc.tile_pool(name="const", bufs=1) as const,
            tc.tile_pool(name="xin", bufs=3) as xin,
            tc.tile_pool(name="adj", bufs=3) as adjp,
            tc.tile_pool(name="act", bufs=3) as actp,
            tc.tile_pool(name="psY", bufs=1, space="PSUM") as psY,
            tc.tile_pool(name="ps256", bufs=2, space="PSUM") as ps256,
            tc.tile_pool(name="ps128", bufs=2, space="PSUM") as ps128,
            tc.tile_pool(name="psRd", bufs=1, space="PSUM") as psRd,
        ):
            # ---- constants -------------------------------------------------
            identity_f32 = const.tile([128, 128], F32)
            make_identity(nc, identity_f32[:])
            identity = const.tile([128, 128], MM_DT)
            nc.vector.tensor_copy(identity[:], identity_f32[:])
            ones_row_f32 = const.tile([1, 128], F32)
            nc.gpsimd.memset(ones_row_f32[:], 1.0)
            ones_row = const.tile([1, 128], MM_DT)
            nc.vector.tensor_copy(ones_row[:], ones_row_f32[:])

            w1p_sb = const.tile([128, KCH, 2 * H_FEATS], MM_DT)
            nc.sync.dma_start(w1p_sb[:], w1p.rearrange("(ko p) n -> p ko n", p=128))
            w2rp_sb = const.tile([128, KCH, 2 * H_FEATS], MM_DT)
            nc.sync.dma_start(w2rp_sb[:], w2rp.rearrange("(ko p) n -> p ko n", p=128))
            xrootst_sb = const.tile([128, KCH, G_PER_CORE], MM_DT)
            nc.sync.dma_start(xrootst_sb[:],
                              xrootst.rearrange("(ko p) n -> p ko n", p=128))
            meansel_sb = const.tile([128, G_PER_CORE * G_PER_CORE], MM_DT)
            nc.sync.dma_start(meansel_sb[:], meansel)
            w2h_sb = {}
            for b in (0, 1):
                t = const.tile([128, 2, H_FEATS], MM_DT, tag=f"w2h_{b}")
                nc.sync.dma_start(t[:], W2H[b].rearrange("(ko p) n -> p ko n", p=128))
                w2h_sb[b] = t
            bias_sb = {}
            if has_bias:
                for nm_, ap_ in (("b1c_td", b1c_td), ("b1c_bu", b1c_bu)):
                    t = const.tile([128, 2], F32, tag=nm_)
                    nc.sync.dma_start(t[:], ap_)
                    bias_sb[nm_] = t
                for nm_, ap_ in (("b2b_td", b2b_td), ("b2b_bu", b2b_bu)):
                    t = const.tile([128, H_FEATS], F32, tag=nm_)
                    nc.sync.dma_start(t[:], ap_)
                    bias_sb[nm_] = t

            # root h collection + per-branch mean accumulators
            roots_sb, psread = {}, {}
            for b in (0, 1):
                rts = const.tile([128, 2, G_PER_CORE], MM_DT, tag=f"roots_{b}")
                roots_sb[b] = rts
                prd = psRd.tile([G_PER_CORE, H_FEATS], F32, tag=f"psread_{b}")
                psread[b] = prd

            # ---- rvec = Xroots @ [W2r_td | W2r_bu]  -> [32, 512] -----------
            ps_rv = psRd.tile([G_PER_CORE, 2 * H_FEATS], F32, tag="ps_small")
            for k in range(KCH):
                nc.tensor.matmul(ps_rv[:], xrootst_sb[:, k, :], w2rp_sb[:, k, :],
                                 start=(k == 0), stop=(k == KCH - 1))
            rvec_sb = const.tile([G_PER_CORE, 2 * H_FEATS], MM_DT)
            nc.vector.tensor_copy(rvec_sb[:], ps_rv[:])
            # flatten to one partition so per-graph rows can be matmul rhs
            rv_dram = nc.dram_tensor("rv_scratch", [G_PER_CORE, 2 * H_FEATS],
                                     MM_DT).ap()
            nc.sync.dma_start(rv_dram[:], rvec_sb[:])
            rvec_row = const.tile([1, G_PER_CORE * 2 * H_FEATS], MM_DT)
            nc.sync.dma_start(rvec_row[:],
                              rv_dram.rearrange("g f -> (g f)")[None, :])

            xt_re = xt.rearrange("(ko p) n -> p ko n", p=128)
            adjt_re = adjt.rearrange("p (g b n) -> p g b n", g=G_PER_CORE, b=2)

            # ---- main loop over this core's graphs -------------------------
            for g in range(G_PER_CORE):
                xt_tile = xin.tile([128, KCH, 128], MM_DT, tag="xt_tile")
                nc.sync.dma_start(xt_tile[:], xt_re[:, :, g * 128:(g + 1) * 128])
                adj_g = adjp.tile([128, 2, 128], MM_DT, tag="adj_g")
                nc.sync.dma_start(adj_g[:], adjt_re[:, g, :, :])

                # Y = X @ [W1_td | W1_bu]  -> [128, 512]
                ps_y = psY.tile([128, 2 * H_FEATS], F32, tag="psY")
                for k in range(KCH):
                    nc.tensor.matmul(ps_y[:], xt_tile[:, k, :], w1p_sb[:, k, :],
                                     start=(k == 0), stop=(k == KCH - 1))
                ysb = actp.tile([128, 2 * H_FEATS], MM_DT, tag="ysb")
                nc.vector.tensor_copy(ysb[:], ps_y[:])

                for b in (0, 1):
                    # hT[f, d] = sum_s Y[s, f] * AnT[s, d]   (2 feat chunks)
                    ht = actp.tile([128, 2, 128], MM_DT, tag="ht")
                    for j in (0, 1):
                        ps_h = ps128.tile([128, 128], F32, tag="ps128")
                        nc.tensor.matmul(
                            ps_h[:], ysb[:, b * H_FEATS + j * 128:
                                         b * H_FEATS + (j + 1) * 128],
                            adj_g[:, b, :])
                        bia = (bias_sb["b1c_td" if b == 0 else "b1c_bu"][:, j:j + 1]
                               if has_bias else 0.0)
                        nc.scalar.activation(ht[:, j, :], ps_h[:], AF.Relu,
                                             bias=bia)
                        # collect root column (node 0)
                        nc.gpsimd.tensor_copy(roots_sb[b][:, j, g:g + 1],
                                              ht[:, j, 0:1])

                    # Z = h @ W2h + ones (x) rvec
                    ps_z = ps256.tile([128, H_FEATS], F32, tag="ps256")
                    nc.tensor.matmul(ps_z[:], ht[:, 0, :], w2h_sb[b][:, 0, :],
                                     start=True, stop=False)
                    nc.tensor.matmul(ps_z[:], ht[:, 1, :], w2h_sb[b][:, 1, :],
                                     start=False, stop=False)
                    rv_off = g * 2 * H_FEATS + b * H_FEATS
                    nc.tensor.matmul(ps_z[:], ones_row[:],
                                     rvec_row[0:1, rv_off:rv_off + H_FEATS],
                                     start=False, stop=True)
                    zn = actp.tile([128, H_FEATS], MM_DT, tag="zn")
                    nc.vector.tensor_copy(zn[:], ps_z[:])

                    # H2 = relu(AnT.T @ Z (+ b2))
                    ps2 = ps256.tile([128, H_FEATS], F32, tag="ps256")
                    nc.tensor.matmul(ps2[:], adj_g[:, b, :], zn[:])
                    h2 = actp.tile([128, H_FEATS], MM_DT, tag="h2")
                    if has_bias:
                        h2f = actp.tile([128, H_FEATS], F32, tag="h2f")
                        nc.vector.tensor_add(
                            h2f[:], ps2[:],
                            bias_sb["b2b_td" if b == 0 else "b2b_bu"][:])
                        nc.scalar.activation(h2[:], h2f[:], AF.Relu)
                    else:
                        nc.scalar.activation(h2[:], ps2[:], AF.Relu)

                    # mean over nodes, accumulated into row g of psread[b]
                    nc.tensor.matmul(
                        psread[b][:],
                        meansel_sb[:, g * G_PER_CORE:(g + 1) * G_PER_CORE],
                        h2[:], start=(g == 0), stop=(g == G_PER_CORE - 1))

            # ---- readout assembly -----------------------------------------
            for b in (0, 1):
                mrows = actp.tile([G_PER_CORE, H_FEATS], F32, tag="mrows")
                nc.scalar.copy(mrows[:], psread[b][:])
                nc.sync.dma_start(out[:, b * 512: b * 512 + 256], mrows[:])
                rootsT = actp.tile([G_PER_CORE, 2, 128], F32, tag="rootsT")
                for j in (0, 1):
                    ps_rt = ps128.tile([G_PER_CORE, 128], MM_DT,
                                       padded_shape=[128, 256], tag="ps128")
                    nc.tensor.transpose(ps_rt[:], roots_sb[b][:, j, :],
                                        identity[:])
                    nc.scalar.copy(rootsT[:, j, :], ps_rt[:])
                nc.sync.dma_start(out[:, b * 512 + 256:(b + 1) * 512],
                                  rootsT[:])

    nc.compile()
    return nc


# ----------------------------------------------------------------------------
# Host entry point
# ----------------------------------------------------------------------------

def _prep(inputs, w1_td, b1_td, w2_td, b2_td, w1_bu, b1_bu, w2_bu, b2_bu,
          td_src, td_dst, bu_src, bu_dst, nodes_per_graph):
    n = int(nodes_per_graph)
    X = np.ascontiguousarray(np.asarray(inputs, np.float32))
    N = X.shape[0]
    G = N // n
    assert (n, G, X.shape[1]) == (N_PER_G, N_GRAPHS, IN_FEATS), \
        f"unexpected shapes {X.shape} n={n}"

    adjt_td = build_adjt(td_src, td_dst, n, G)  # [G, s, d] f32
    adjt_bu = build_adjt(bu_src, bu_dst, n, G)
    # [G, 2, s, d] -> per-core [s, g, b, d] flattened
    adjt = np.stack([adjt_td, adjt_bu], axis=1)

    w1p = np.ascontiguousarray(
        np.concatenate([np.asarray(w1_td, np.float32),
                        np.asarray(w1_bu, np.float32)], axis=1))
    w2_td = np.asarray(w2_td, np.float32)
    w2_bu = np.asarray(w2_bu, np.float32)
    w2rp = np.ascontiguousarray(
        np.concatenate([w2_td[H_FEATS:], w2_bu[H_FEATS:]], axis=1))
    biases = [np.asarray(b, np.float32) for b in (b1_td, b2_td, b1_bu, b2_bu)]
    has_bias = any(np.any(b != 0) for b in biases)

    msel = np.zeros((128, G_PER_CORE, G_PER_CORE), np.float32)
    for gi in range(G_PER_CORE):
        msel[:, gi, gi] = 1.0 / N_PER_G
    msel = msel.reshape(128, G_PER_CORE * G_PER_CORE).astype(BF16)

    Xbf = X.astype(BF16)
    in_maps = []
    for c in range(N_CORES):
        gs = slice(c * G_PER_CORE, (c + 1) * G_PER_CORE)
        ns = slice(c * NODES_PER_CORE, (c + 1) * NODES_PER_CORE)
        Xc = Xbf[ns]
        adc = adjt[gs]  # [32, 2, 128, 128]
        m = {
            "xt": np.ascontiguousarray(Xc.T),
            "xrootst": np.ascontiguousarray(Xc[::n].T),
            "adjt": np.ascontiguousarray(
                adc.transpose(2, 0, 1, 3).reshape(128, -1)).astype(BF16),
            "w1p": w1p.astype(BF16),
            "w2h_td": np.ascontiguousarray(w2_td[:H_FEATS]).astype(BF16),
            "w2h_bu": np.ascontiguousarray(w2_bu[:H_FEATS]).astype(BF16),
            "w2rp": w2rp.astype(BF16),
            "meansel": msel,
        }
        if has_bias:
            m["b1c_td"] = np.ascontiguousarray(
                biases[0].reshape(2, 128).T.astype(np.float32))
            m["b1c_bu"] = np.ascontiguousarray(
                biases[2].reshape(2, 128).T.astype(np.float32))
            m["b2b_td"] = np.ascontiguousarray(
                np.broadcast_to(biases[1], (128, H_FEATS)).astype(np.float32))
            m["b2b_bu"] = np.ascontiguousarray(
                np.broadcast_to(biases[3], (128, H_FEATS)).astype(np.float32))
        in_maps.append(m)
    return in_maps, has_bias


_PROGRAM_CACHE = {}


def _get_program(key):
    if key not in _PROGRAM_CACHE:
        _PROGRAM_CACHE[key] = build_program(key)
    return _PROGRAM_CACHE[key]


def kernel(trace=False, tmpdir=None, _return_raw=False, **inputs):
    in_maps, has_bias = _prep(**inputs)
    nc = _get_program(has_bias)
    res = run_bass_kernel_spmd(nc, in_maps, list(range(N_CORES)),
                               trace=trace, tmpdir=tmpdir)
    out = np.concatenate([res.results[i]["out"] for i in range(N_CORES)], axis=0)
    if _return_raw:
        return out, res
    return out


# revision 4
# speedup vs baseline: 1.1453x; 1.1453x over previous
"""BiGCN (bidirectional 2-layer GCN over many small graphs) on 8 Trainium2 cores.

Strategy: data-parallel over graphs (32 graphs of 128 nodes per core). The
host precomputes each graph's dense *normalized* adjacency
  An = D^{-1/2} (A + I) D^{-1/2}
(transposed, [src, dst] layout) so the device does only dense bf16 matmuls:

  per graph (n=128 nodes), per branch:
    Y   = X @ [W1_td | W1_bu]                  (shared across branches)
    hT  = relu( Y_b^T-contracted: matmul(lhsT=Y_b chunk, rhs=AnT) )  [feat, node]
    Z   = h @ W2h + ones (x) rvec              (rank-1 fold of the root term)
    H2  = relu( matmul(lhsT=AnT, rhs=Z) )      [node, feat]
    readout: mean over nodes via PSUM-accumulated selector matmul,
             root row collected from hT column 0, transposed once at the end.
Final output: concat(TD branch, BU branch) -> [G, 1024].
"""

import numpy as np
import ml_dtypes

import concourse.bass as bass
import concourse.tile as tile
from concourse import bacc, mybir
from concourse.bass_utils import run_bass_kernel_spmd
from concourse.masks import make_identity

# Problem shape (fixed by the task)
N_GRAPHS = 256
N_PER_G = 128
IN_FEATS = 768
H_FEATS = 256
N_CORES = 8
G_PER_CORE = N_GRAPHS // N_CORES            # 32
NODES_PER_CORE = G_PER_CORE * N_PER_G       # 4096
KCH = IN_FEATS // 128                       # 6 feature chunks

MM_DT = mybir.dt.bfloat16
BF16 = ml_dtypes.bfloat16
F32 = mybir.dt.float32
AF = mybir.ActivationFunctionType
OP = mybir.AluOpType


# ----------------------------------------------------------------------------
# Host-side prep: dense normalized adjacency per graph
# ----------------------------------------------------------------------------

def build_adjt(src, dst, n, G):
    """AnT[g, s, d] = norm[d] * A[d, s] * norm[s], A[d, s] = #edges s->d
    (self-loops included in the edge list)."""
    src = np.asarray(src, np.int64)
    dst = np.asarray(dst, np.int64)
    g = dst // n
    if not np.array_equal(src // n, g):
        raise ValueError("cross-graph edge found; contiguous-block sharding invalid")
    A = np.zeros((G, n, n), np.float32)
    np.add.at(A, (g, dst % n, src % n), 1.0)
    deg = A.sum(axis=2)  # in-degree (incl. self-loops); >= 1 by construction
    norm = 1.0 / np.sqrt(deg)
    An = norm[:, :, None] * A * norm[:, None, :]
    return An.transpose(0, 2, 1)  # [G, s, d]


# ----------------------------------------------------------------------------
# Device program (SPMD; one core's shard)
# ----------------------------------------------------------------------------

def build_program(has_bias):
    nc = bacc.Bacc("TRN2", target_bir_lowering=False, debug=False,
                   num_devices=N_CORES)

    def din(name, shape, dt=MM_DT):
        return nc.dram_tensor(name, shape, dt, kind="ExternalInput").ap()

    xt = din("xt", [IN_FEATS, NODES_PER_CORE])
    adjt = din("adjt", [128, G_PER_CORE * 2 * 128])
    xrootst = din("xrootst", [IN_FEATS, G_PER_CORE])
    w1p = din("w1p", [IN_FEATS, 2 * H_FEATS])
    w2h_td = din("w2h_td", [H_FEATS, H_FEATS])
    w2h_bu = din("w2h_bu", [H_FEATS, H_FEATS])
    w2rp = din("w2rp", [IN_FEATS, 2 * H_FEATS])
    meansel = din("meansel", [128, G_PER_CORE * G_PER_CORE])
    if has_bias:
        b1c_td = din("b1c_td", [128, 2], F32)   # b1 as column chunks [feat,1]
        b1c_bu = din("b1c_bu", [128, 2], F32)
        b2b_td = din("b2b_td", [128, H_FEATS], F32)  # b2 broadcast over nodes
        b2b_bu = din("b2b_bu", [128, H_FEATS], F32)
    out = nc.dram_tensor("out", [G_PER_CORE, 4 * H_FEATS], F32,
                         kind="ExternalOutput").ap()

    W2H = {0: w2h_td, 1: w2h_bu}

    with tile.TileContext(nc) as tc:
        with (
            tc.tile_pool(name="const", bufs=1) as const,
            tc.tile_pool(name="xin", bufs=4) as xin,
            tc.tile_pool(name="adj", bufs=4) as adjp,
            tc.tile_pool(name="act", bufs=6) as actp,
            tc.tile_pool(name="psY", bufs=2, space="PSUM") as psY,
            tc.tile_pool(name="psZ", bufs=2, space="PSUM") as psZ,
            tc.tile_pool(name="psH2", bufs=2, space="PSUM") as psH2,
            tc.tile_pool(name="ps128", bufs=2, space="PSUM") as ps128,
        ):
            # ---- constants -------------------------------------------------
            identity_f32 = const.tile([128, 128], F32)
            make_identity(nc, identity_f32[:])
            identity = const.tile([128, 128], MM_DT)
            nc.vector.tensor_copy(identity[:], identity_f32[:])
            ones_row_f32 = const.tile([1, 128], F32)
            nc.gpsimd.memset(ones_row_f32[:], 1.0)
            ones_row = const.tile([1, 128], MM_DT)
            nc.vector.tensor_copy(ones_row[:], ones_row_f32[:])

            w1p_sb = const.tile([128, KCH, 2 * H_FEATS], MM_DT)
            nc.sync.dma_start(w1p_sb[:], w1p.rearrange("(ko p) n -> p ko n", p=128))
            w2rp_sb = const.tile([128, KCH, 2 * H_FEATS], MM_DT)
            nc.sync.dma_start(w2rp_sb[:], w2rp.rearrange("(ko p) n -> p ko n", p=128))
            xrootst_sb = const.tile([128, KCH, G_PER_CORE], MM_DT)
            nc.sync.dma_start(xrootst_sb[:],
                              xrootst.rearrange("(ko p) n -> p ko n", p=128))
            meansel_sb = const.tile([128, G_PER_CORE * G_PER_CORE], MM_DT)
            nc.sync.dma_start(meansel_sb[:], meansel)
            w2h_sb = {}
            for b in (0, 1):
                t = const.tile([128, 2, H_FEATS], MM_DT, tag=f"w2h_{b}")
                nc.sync.dma_start(t[:], W2H[b].rearrange("(ko p) n -> p ko n", p=128))
                w2h_sb[b] = t
            bias_sb = {}
            if has_bias:
                for nm_, ap_ in (("b1c_td", b1c_td), ("b1c_bu", b1c_bu)):
                    t = const.tile([128, 2], F32, tag=nm_)
                    nc.sync.dma_start(t[:], ap_)
                    bias_sb[nm_] = t
                for nm_, ap_ in (("b2b_td", b2b_td), ("b2b_bu", b2b_bu)):
                    t = const.tile([128, H_FEATS], F32, tag=nm_)
                    nc.sync.dma_start(t[:], ap_)
                    bias_sb[nm_] = t

            # root h collection + per-branch H2 stash (mean is computed in a
            # short tail phase so no PSUM bank is held across the main loop)
            roots_sb, h2_all = {}, {}
            for b in (0, 1):
                rts = const.tile([128, 2, G_PER_CORE], MM_DT, tag=f"roots_{b}")
                roots_sb[b] = rts
                h2a = const.tile([128, G_PER_CORE, H_FEATS], MM_DT,
                                 tag=f"h2_all_{b}")
                h2_all[b] = h2a

            # ---- rvec = Xroots @ [W2r_td | W2r_bu]  -> [32, 512] -----------
            ps_rv = psY.tile([G_PER_CORE, 2 * H_FEATS], F32,
                             padded_shape=[128, 2 * H_FEATS], tag="psY")
            for k in range(KCH):
                nc.tensor.matmul(ps_rv[:], xrootst_sb[:, k, :], w2rp_sb[:, k, :],
                                 start=(k == 0), stop=(k == KCH - 1))
            rvec_sb = const.tile([G_PER_CORE, 2 * H_FEATS], MM_DT)
            nc.vector.tensor_copy(rvec_sb[:], ps_rv[:])
            # flatten to one partition so per-graph rows can be matmul rhs
            rv_dram = nc.dram_tensor("rv_scratch", [G_PER_CORE, 2 * H_FEATS],
                                     MM_DT).ap()
            nc.sync.dma_start(rv_dram[:], rvec_sb[:])
            rvec_row = const.tile([1, G_PER_CORE * 2 * H_FEATS], MM_DT)
            nc.sync.dma_start(rvec_row[:],
                              rv_dram.rearrange("g f -> (g f)")[None, :])

            xt_re = xt.rearrange("(ko p) n -> p ko n", p=128)
            adjt_re = adjt.rearrange("p (g b n) -> p g b n", g=G_PER_CORE, b=2)

            # ---- main loop over this core's graphs (software-pipelined) ----
            def emit_in(g):
                xt_tile = xin.tile([128, KCH, 128], MM_DT, tag="xt_tile",
                                   name=f"xt_{g}")
                nc.sync.dma_start(xt_tile[:], xt_re[:, :, g * 128:(g + 1) * 128])
                adj_g = adjp.tile([128, 2, 128], MM_DT, tag="adj_g",
                                  name=f"adj_{g}")
                nc.sync.dma_start(adj_g[:], adjt_re[:, g, :, :])
                return xt_tile, adj_g

            def emit_y(g, xt_tile):
                # Y = X @ [W1_td | W1_bu]  -> [128, 512]
                ps_y = psY.tile([128, 2 * H_FEATS], F32, tag="psY",
                                name=f"ps_y_{g}")
                for k in range(KCH):
                    nc.tensor.matmul(ps_y[:], xt_tile[:, k, :], w1p_sb[:, k, :],
                                     start=(k == 0), stop=(k == KCH - 1))
                return ps_y

            inflight = {0: emit_in(0)}
            ps_y_cur = emit_y(0, inflight[0][0])
            inflight[1] = emit_in(1)

            for g in range(G_PER_CORE):
                adj_g = inflight[g][1]
                ysb = actp.tile([128, 2 * H_FEATS], MM_DT, tag="ysb",
                                name=f"ysb_{g}")
                # split the PSUM->SBUF cast per branch so hT can start earlier
                nc.vector.tensor_copy(ysb[:, 0:H_FEATS], ps_y_cur[:, 0:H_FEATS])
                nc.vector.tensor_copy(ysb[:, H_FEATS:], ps_y_cur[:, H_FEATS:])

                # overlap: next graph's Y matmuls + the DMA two graphs ahead
                if g + 1 < G_PER_CORE:
                    ps_y_cur = emit_y(g + 1, inflight[g + 1][0])
                if g + 2 < G_PER_CORE:
                    inflight[g + 2] = emit_in(g + 2)
                inflight.pop(g - 1, None)

                # layer 1 for both branches, interleaved
                ht = {}
                for b in (0, 1):
                    htb = actp.tile([128, 2, 128], MM_DT, tag="ht",
                                    name=f"ht_{g}_{b}")
                    ht[b] = htb
                    for j in (0, 1):
                        ps_h = ps128.tile([128, 128], F32, tag="ps128",
                                          name=f"ps_h_{g}_{b}_{j}")
                        nc.tensor.matmul(
                            ps_h[:], ysb[:, b * H_FEATS + j * 128:
                                         b * H_FEATS + (j + 1) * 128],
                            adj_g[:, b, :])
                        bia = (bias_sb["b1c_td" if b == 0 else "b1c_bu"][:, j:j + 1]
                               if has_bias else 0.0)
                        if j == 0:
                            nc.scalar.activation(htb[:, j, :], ps_h[:], AF.Relu,
                                                 bias=bia)
                        elif has_bias:
                            nc.vector.tensor_scalar(htb[:, j, :], ps_h[:], bia,
                                                    0.0, OP.add, OP.max)
                        else:
                            nc.vector.tensor_scalar(htb[:, j, :], ps_h[:], 0.0,
                                                    None, OP.max)
                        # collect root column (node 0); gpsimd is idle and
                        # this is SBUF->SBUF (gpsimd has no PSUM port)
                        nc.gpsimd.tensor_copy(roots_sb[b][:, j, g:g + 1],
                                              htb[:, j, 0:1])

                # Z = h @ W2h + ones (x) rvec, both branches
                zn = {}
                for b in (0, 1):
                    ps_z = psZ.tile([128, H_FEATS], F32, tag="psZ",
                                    name=f"ps_z_{g}_{b}")
                    nc.tensor.matmul(ps_z[:], ht[b][:, 0, :], w2h_sb[b][:, 0, :],
                                     start=True, stop=False)
                    nc.tensor.matmul(ps_z[:], ht[b][:, 1, :], w2h_sb[b][:, 1, :],
                                     start=False, stop=False)
                    rv_off = g * 2 * H_FEATS + b * H_FEATS
                    nc.tensor.matmul(ps_z[:], ones_row[:],
                                     rvec_row[0:1, rv_off:rv_off + H_FEATS],
                                     start=False, stop=True)
                    znb = actp.tile([128, H_FEATS], MM_DT, tag="zn",
                                    name=f"zn_{g}_{b}")
                    zn[b] = znb
                    if b == 0:
                        nc.vector.tensor_copy(znb[:], ps_z[:])
                    else:
                        nc.scalar.copy(znb[:], ps_z[:])

                # H2 = relu(AnT.T @ Z (+ b2)), stashed for the mean tail
                for b in (0, 1):
                    ps2 = psH2.tile([128, H_FEATS], F32, tag="psH2",
                                    name=f"ps2_{g}_{b}")
                    nc.tensor.matmul(ps2[:], adj_g[:, b, :], zn[b][:])
                    if has_bias:
                        h2f = actp.tile([128, H_FEATS], F32, tag="h2f",
                                        name=f"h2f_{g}_{b}")
                        nc.vector.tensor_add(
                            h2f[:], ps2[:],
                            bias_sb["b2b_td" if b == 0 else "b2b_bu"][:])
                        nc.scalar.activation(h2_all[b][:, g, :], h2f[:],
                                             AF.Relu)
                    else:
                        nc.scalar.activation(h2_all[b][:, g, :], ps2[:],
                                             AF.Relu)

            # ---- readout assembly -----------------------------------------
            # mean tail: one matmul per PAIR of graphs (512-wide moving),
            # row gp of psread = [mean(2gp) | mean(2gp+1)]
            NPAIR = G_PER_CORE // 2
            for b in (0, 1):
                psread = psY.tile([G_PER_CORE, 2 * H_FEATS], F32,
                                  padded_shape=[128, 2 * H_FEATS], tag="psY",
                                  name=f"psread_{b}")
                for gp in range(NPAIR):
                    nc.tensor.matmul(
                        psread[:],
                        meansel_sb[:, gp * G_PER_CORE:(gp + 1) * G_PER_CORE],
                        h2_all[b][:, 2 * gp:2 * gp + 2, :],
                        start=(gp == 0), stop=(gp == NPAIR - 1))
                mrows = actp.tile([NPAIR, 2, H_FEATS], F32, tag="mrows",
                                  name=f"mrows_{b}")
                nc.scalar.copy(mrows[:], psread[0:NPAIR, :].rearrange(
                    "p (two f) -> p two f", two=2))
                nc.sync.dma_start(
                    out[:, b * 512: b * 512 + 256].rearrange(
                        "(gp two) f -> gp two f", two=2),
                    mrows[:])
                rootsT = actp.tile([G_PER_CORE, 2, 128], F32, tag="rootsT",
                                   name=f"rootsT_{b}")
                for j in (0, 1):
                    ps_rt = ps128.tile([G_PER_CORE, 128], MM_DT,
                                       padded_shape=[128, 256], tag="ps128",
                                       name=f"ps_rt_{b}_{j}")
                    nc.tensor.transpose(ps_rt[:], roots_sb[b][:, j, :],
                                        identity[:])
                    nc.scalar.copy(rootsT[:, j, :], ps_rt[:])
                nc.sync.dma_start(out[:, b * 512 + 256:(b + 1) * 512],
                                  rootsT[:])

    nc.compile()
    return nc


# ----------------------------------------------------------------------------
# Host entry point
# ----------------------------------------------------------------------------

def _prep(inputs, w1_td, b1_td, w2_td, b2_td, w1_bu, b1_bu, w2_bu, b2_bu,
          td_src, td_dst, bu_src, bu_dst, nodes_per_graph):
    n = int(nodes_per_graph)
    X = np.ascontiguousarray(np.asarray(inputs, np.float32))
    N = X.shape[0]
    G = N // n
    assert (n, G, X.shape[1]) == (N_PER_G, N_GRAPHS, IN_FEATS), \
        f"unexpected shapes {X.shape} n={n}"

    adjt_td = build_adjt(td_src, td_dst, n, G)  # [G, s, d] f32
    adjt_bu = build_adjt(bu_src, bu_dst, n, G)
    # [G, 2, s, d] -> per-core [s, g, b, d] flattened
    adjt = np.stack([adjt_td, adjt_bu], axis=1)

    w1p = np.ascontiguousarray(
        np.concatenate([np.asarray(w1_td, np.float32),
                        np.asarray(w1_bu, np.float32)], axis=1))
    w2_td = np.asarray(w2_td, np.float32)
    w2_bu = np.asarray(w2_bu, np.float32)
    w2rp = np.ascontiguousarray(
        np.concatenate([w2_td[H_FEATS:], w2_bu[H_FEATS:]], axis=1))
    biases = [np.asarray(b, np.float32) for b in (b1_td, b2_td, b1_bu, b2_bu)]
    has_bias = any(np.any(b != 0) for b in biases)

    msel = np.zeros((128, G_PER_CORE, G_PER_CORE), np.float32)
    for gi in range(G_PER_CORE):
        msel[:, gi, gi] = 1.0 / N_PER_G
    msel = msel.reshape(128, G_PER_CORE * G_PER_CORE).astype(BF16)

    Xbf = X.astype(BF16)
    in_maps = []
    for c in range(N_CORES):
        gs = slice(c * G_PER_CORE, (c + 1) * G_PER_CORE)
        ns = slice(c * NODES_PER_CORE, (c + 1) * NODES_PER_CORE)
        Xc = Xbf[ns]
        adc = adjt[gs]  # [32, 2, 128, 128]
        m = {
            "xt": np.ascontiguousarray(Xc.T),
            "xrootst": np.ascontiguousarray(Xc[::n].T),
            "adjt": np.ascontiguousarray(
                adc.transpose(2, 0, 1, 3).reshape(128, -1)).astype(BF16),
            "w1p": w1p.astype(BF16),
            "w2h_td": np.ascontiguousarray(w2_td[:H_FEATS]).astype(BF16),
            "w2h_bu": np.ascontiguousarray(w2_bu[:H_FEATS]).astype(BF16),
            "w2rp": w2rp.astype(BF16),
            "meansel": msel,
        }
        if has_bias:
            m["b1c_td"] = np.ascontiguousarray(
                biases[0].reshape(2, 128).T.astype(np.float32))
            m["b1c_bu"] = np.ascontiguousarray(
                biases[2].reshape(2, 128).T.astype(np.float32))
            m["b2b_td"] = np.ascontiguousarray(
                np.broadcast_to(biases[1], (128, H_FEATS)).astype(np.float32))
            m["b2b_bu"] = np.ascontiguousarray(
                np.broadcast_to(biases[3], (128, H_FEATS)).astype(np.float32))
        in_maps.append(m)
    return in_maps, has_bias


_PROGRAM_CACHE = {}


def _get_program(key):
    if key not in _PROGRAM_CACHE:
        _PROGRAM_CACHE[key] = build_program(key)
    return _PROGRAM_CACHE[key]


def kernel(trace=False, tmpdir=None, _return_raw=False, **inputs):
    in_maps, has_bias = _prep(**inputs)
    nc = _get_program(has_bias)
    res = run_bass_kernel_spmd(nc, in_maps, list(range(N_CORES)),
                               trace=trace, tmpdir=tmpdir)
    out = np.concatenate([res.results[i]["out"] for i in range(N_CORES)], axis=0)
    if _return_raw:
        return out, res
    return out


# revision 6
# speedup vs baseline: 1.2235x; 1.0683x over previous
"""BiGCN (bidirectional 2-layer GCN over many small graphs) on 8 Trainium2 cores.

Strategy: data-parallel over graphs (32 graphs of 128 nodes per core). The
host precomputes each graph's dense *normalized* adjacency
  An = D^{-1/2} (A + I) D^{-1/2}
(transposed, [src, dst] layout) so the device does only dense bf16 matmuls:

  per graph (n=128 nodes), per branch:
    Y   = X @ [W1_td | W1_bu]                  (shared across branches)
    hT  = relu( Y_b^T-contracted: matmul(lhsT=Y_b chunk, rhs=AnT) )  [feat, node]
    Z   = h @ W2h + ones (x) rvec              (rank-1 fold of the root term)
    H2  = relu( matmul(lhsT=AnT, rhs=Z) )      [node, feat]
    readout: mean over nodes via PSUM-accumulated selector matmul,
             root row collected from hT column 0, transposed once at the end.
Final output: concat(TD branch, BU branch) -> [G, 1024].
"""

import numpy as np
import ml_dtypes

import concourse.bass as bass
import concourse.tile as tile
from concourse import bacc, mybir
from concourse.bass_utils import run_bass_kernel_spmd
from concourse.masks import make_identity

# Problem shape (fixed by the task)
N_GRAPHS = 256
N_PER_G = 128
IN_FEATS = 768
H_FEATS = 256
N_CORES = 8
G_PER_CORE = N_GRAPHS // N_CORES            # 32
NODES_PER_CORE = G_PER_CORE * N_PER_G       # 4096
KCH = IN_FEATS // 128                       # 6 feature chunks

MM_DT = mybir.dt.bfloat16
BF16 = ml_dtypes.bfloat16
F32 = mybir.dt.float32
AF = mybir.ActivationFunctionType
OP = mybir.AluOpType


# ----------------------------------------------------------------------------
# Host-side prep: dense normalized adjacency per graph
# ----------------------------------------------------------------------------

def build_adjt(src, dst, n, G):
    """AnT[g, s, d] = norm[d] * A[d, s] * norm[s], A[d, s] = #edges s->d
    (self-loops included in the edge list)."""
    src = np.asarray(src, np.int64)
    dst = np.asarray(dst, np.int64)
    g = dst // n
    if not np.array_equal(src // n, g):
        raise ValueError("cross-graph edge found; contiguous-block sharding invalid")
    A = np.zeros((G, n, n), np.float32)
    np.add.at(A, (g, dst % n, src % n), 1.0)
    deg = A.sum(axis=2)  # in-degree (incl. self-loops); >= 1 by construction
    norm = 1.0 / np.sqrt(deg)
    An = norm[:, :, None] * A * norm[:, None, :]
    return An.transpose(0, 2, 1)  # [G, s, d]


# ----------------------------------------------------------------------------
# Device program (SPMD; one core's shard)
# ----------------------------------------------------------------------------

def build_program(has_bias):
    nc = bacc.Bacc("TRN2", target_bir_lowering=False, debug=False,
                   num_devices=N_CORES)

    def din(name, shape, dt=MM_DT):
        return nc.dram_tensor(name, shape, dt, kind="ExternalInput").ap()

    xt = din("xt", [IN_FEATS, NODES_PER_CORE])
    adjt = din("adjt", [128, G_PER_CORE * 2 * 128])
    w1p = din("w1p", [IN_FEATS, 2 * H_FEATS])
    w2h_td = din("w2h_td", [H_FEATS, H_FEATS])
    w2h_bu = din("w2h_bu", [H_FEATS, H_FEATS])
    rvecp = din("rvecp", [1, G_PER_CORE * 2 * H_FEATS])
    meansel = din("meansel", [128, G_PER_CORE * G_PER_CORE])
    if has_bias:
        b1c_td = din("b1c_td", [128, 2], F32)   # b1 as column chunks [feat,1]
        b1c_bu = din("b1c_bu", [128, 2], F32)
        b2b_td = din("b2b_td", [128, H_FEATS], F32)  # b2 broadcast over nodes
        b2b_bu = din("b2b_bu", [128, H_FEATS], F32)
    out = nc.dram_tensor("out", [G_PER_CORE, 4 * H_FEATS], F32,
                         kind="ExternalOutput").ap()

    W2H = {0: w2h_td, 1: w2h_bu}

    with tile.TileContext(nc) as tc:
        with (
            tc.tile_pool(name="const", bufs=1) as const,
            tc.tile_pool(name="xin", bufs=4) as xin,
            tc.tile_pool(name="adj", bufs=4) as adjp,
            tc.tile_pool(name="act", bufs=6) as actp,
            tc.tile_pool(name="psY", bufs=2, space="PSUM") as psY,
            tc.tile_pool(name="psHT", bufs=2, space="PSUM") as psHT,
            tc.tile_pool(name="psZ", bufs=2, space="PSUM") as psZ,
            tc.tile_pool(name="psH2", bufs=2, space="PSUM") as psH2,
        ):
            # ---- critical-path loads first: xt(0), w1p chunks, adj(0) ------
            xt_re = xt.rearrange("(ko p) n -> p ko n", p=128)
            adjt_re = adjt.rearrange("p (g b n) -> p g b n", g=G_PER_CORE, b=2)
            w1p_re = w1p.rearrange("(ko p) n -> p ko n", p=128)

            xt0 = xin.tile([128, KCH, 128], MM_DT, tag="xt_tile", name="xt_0")
            nc.sync.dma_start(xt0[:], xt_re[:, :, 0:128])
            w1p_sb = const.tile([128, KCH, 2 * H_FEATS], MM_DT)
            qs = [nc.scalar, nc.gpsimd]
            for k in range(KCH):
                qs[k % 2].dma_start(w1p_sb[:, k, :], w1p_re[:, k, :])
            adj0 = adjp.tile([128, 2, 128], MM_DT, tag="adj_g", name="adj_0")
            nc.sync.dma_start(adj0[:], adjt_re[:, 0, :, :])

            # remaining constants on the gpsimd DMA queue (parallel issue)
            identity_f32 = const.tile([128, 128], F32)
            make_identity(nc, identity_f32[:])
            identity = const.tile([128, 128], MM_DT)
            nc.vector.tensor_copy(identity[:], identity_f32[:])
            ones_row_f32 = const.tile([1, 128], F32)
            nc.gpsimd.memset(ones_row_f32[:], 1.0)
            ones_row = const.tile([1, 128], MM_DT)
            nc.vector.tensor_copy(ones_row[:], ones_row_f32[:])

            rvec_row = const.tile([1, G_PER_CORE * 2 * H_FEATS], MM_DT)
            nc.gpsimd.dma_start(rvec_row[:], rvecp)
            meansel_sb = const.tile([128, G_PER_CORE * G_PER_CORE], MM_DT)
            nc.gpsimd.dma_start(meansel_sb[:], meansel)
            w2h_sb = {}
            for b in (0, 1):
                t = const.tile([128, 2, H_FEATS], MM_DT, tag=f"w2h_{b}")
                nc.gpsimd.dma_start(t[:], W2H[b].rearrange("(ko p) n -> p ko n",
                                                           p=128))
                w2h_sb[b] = t
            bias_sb = {}
            if has_bias:
                for nm_, ap_ in (("b1c_td", b1c_td), ("b1c_bu", b1c_bu)):
                    t = const.tile([128, 2], F32, tag=nm_)
                    nc.gpsimd.dma_start(t[:], ap_)
                    bias_sb[nm_] = t
                for nm_, ap_ in (("b2b_td", b2b_td), ("b2b_bu", b2b_bu)):
                    t = const.tile([128, H_FEATS], F32, tag=nm_)
                    nc.gpsimd.dma_start(t[:], ap_)
                    bias_sb[nm_] = t

            # root h collection + H2 stash (mean is computed in a short tail
            # phase so no PSUM bank is held across the main loop)
            roots_sb = {}
            for b in (0, 1):
                rts = const.tile([128, 2, G_PER_CORE], MM_DT, tag=f"roots_{b}")
                roots_sb[b] = rts
            h2_all = const.tile([128, G_PER_CORE, 2, H_FEATS], MM_DT)

            # ---- main loop over this core's graphs (software-pipelined) ----
            def emit_in(g):
                xt_tile = xin.tile([128, KCH, 128], MM_DT, tag="xt_tile",
                                   name=f"xt_{g}")
                nc.sync.dma_start(xt_tile[:], xt_re[:, :, g * 128:(g + 1) * 128])
                adj_g = adjp.tile([128, 2, 128], MM_DT, tag="adj_g",
                                  name=f"adj_{g}")
                nc.sync.dma_start(adj_g[:], adjt_re[:, g, :, :])
                return xt_tile, adj_g

            def emit_y(g, xt_tile):
                # Y = X @ [W1_td | W1_bu]  -> [128, 512]
                ps_y = psY.tile([128, 2 * H_FEATS], F32, tag="psY",
                                name=f"ps_y_{g}")
                for k in range(KCH):
                    nc.tensor.matmul(ps_y[:], xt_tile[:, k, :], w1p_sb[:, k, :],
                                     start=(k == 0), stop=(k == KCH - 1))
                return ps_y

            inflight = {0: (xt0, adj0)}
            ps_y_cur = emit_y(0, xt0)
            inflight[1] = emit_in(1)

            for g in range(G_PER_CORE):
                adj_g = inflight[g][1]
                ysb = actp.tile([128, 2 * H_FEATS], MM_DT, tag="ysb",
                                name=f"ysb_{g}")
                nc.vector.tensor_copy(ysb[:], ps_y_cur[:])

                # overlap: next graph's Y matmuls + the DMA two graphs ahead
                if g + 1 < G_PER_CORE:
                    ps_y_cur = emit_y(g + 1, inflight[g + 1][0])
                if g + 2 < G_PER_CORE:
                    inflight[g + 2] = emit_in(g + 2)
                inflight.pop(g - 1, None)

                # layer 1, both branches into one PSUM bank -> single relu
                ht = actp.tile([128, 2, 2, 128], MM_DT, tag="ht",
                               name=f"ht_{g}")
                ps_h = psHT.tile([128, 2, 2, 128], F32, tag="psHT",
                                 name=f"ps_h_{g}")
                for b in (0, 1):
                    for j in (0, 1):
                        nc.tensor.matmul(
                            ps_h[:, b, j, :], ysb[:, b * H_FEATS + j * 128:
                                                  b * H_FEATS + (j + 1) * 128],
                            adj_g[:, b, :])
                if has_bias:
                    for b in (0, 1):
                        bcol = bias_sb["b1c_td" if b == 0 else "b1c_bu"]
                        for j in (0, 1):
                            nc.scalar.activation(ht[:, b, j, :],
                                                 ps_h[:, b, j, :], AF.Relu,
                                                 bias=bcol[:, j:j + 1])
                else:
                    nc.scalar.activation(ht[:, :, :, :], ps_h[:, :, :, :],
                                         AF.Relu)
                for b in (0, 1):
                    for j in (0, 1):
                        # collect root column (node 0); gpsimd is idle and
                        # this is SBUF->SBUF (gpsimd has no PSUM port)
                        nc.gpsimd.tensor_copy(roots_sb[b][:, j, g:g + 1],
                                              ht[:, b, j, 0:1])

                # Z = h @ W2h + ones (x) rvec; both branches in one bank,
                # rank-1 root fold done 512-wide across both branches
                ps_z = psZ.tile([128, 2, H_FEATS], F32, tag="psZ",
                                name=f"ps_z_{g}")
                # rank-1 root fold first: its start=True initializes the whole
                # bank, so the per-branch w2h matmuls can accumulate into
                # their halves as one group (a second start=True while the
                # bank has an open group corrupts the open half)
                nc.tensor.matmul(
                    ps_z[:, :, :], ones_row[:],
                    rvec_row[0:1, g * 2 * H_FEATS:(g + 1) * 2 * H_FEATS],
                    start=True, stop=False, skip_group_check=True)
                for b in (0, 1):
                    nc.tensor.matmul(ps_z[:, b, :], ht[:, b, 0, :],
                                     w2h_sb[b][:, 0, :], start=False,
                                     stop=False, skip_group_check=True)
                    nc.tensor.matmul(ps_z[:, b, :], ht[:, b, 1, :],
                                     w2h_sb[b][:, 1, :], start=False,
                                     stop=(b == 1), skip_group_check=True)
                zn = actp.tile([128, 2, H_FEATS], MM_DT, tag="zn",
                               name=f"zn_{g}")
                nc.vector.tensor_copy(zn[:], ps_z[:])

                # H2 = relu(AnT.T @ Z (+ b2)), stashed for the mean tail
                ps2 = psH2.tile([128, 2, H_FEATS], F32, tag="psH2",
                                name=f"ps2_{g}")
                for b in (0, 1):
                    nc.tensor.matmul(ps2[:, b, :], adj_g[:, b, :], zn[:, b, :])
                if has_bias:
                    h2f = actp.tile([128, 2, H_FEATS], F32, tag="h2f",
                                    name=f"h2f_{g}")
                    for b in (0, 1):
                        nc.vector.tensor_add(
                            h2f[:, b, :], ps2[:, b, :],
                            bias_sb["b2b_td" if b == 0 else "b2b_bu"][:])
                    nc.scalar.activation(h2_all[:, g, :, :], h2f[:], AF.Relu)
                else:
                    nc.scalar.activation(h2_all[:, g, :, :], ps2[:], AF.Relu)

            # ---- readout assembly -----------------------------------------
            # mean tail: one matmul per PAIR of graphs (512-wide moving),
            # row gp of psread = [mean(2gp) | mean(2gp+1)]
            NPAIR = G_PER_CORE // 2
            for b in (0, 1):
                psread = psY.tile([G_PER_CORE, 2 * H_FEATS], F32,
                                  padded_shape=[128, 2 * H_FEATS], tag="psY",
                                  name=f"psread_{b}")
                for gp in range(NPAIR):
                    nc.tensor.matmul(
                        psread[:],
                        meansel_sb[:, gp * G_PER_CORE:(gp + 1) * G_PER_CORE],
                        h2_all[:, 2 * gp:2 * gp + 2, b, :],
                        start=(gp == 0), stop=(gp == NPAIR - 1))
                mrows = actp.tile([NPAIR, 2, H_FEATS], F32, tag="mrows",
                                  name=f"mrows_{b}")
                nc.scalar.copy(mrows[:], psread[0:NPAIR, :].rearrange(
                    "p (two f) -> p two f", two=2))
                nc.sync.dma_start(
                    out[:, b * 512: b * 512 + 256].rearrange(
                        "(gp two) f -> gp two f", two=2),
                    mrows[:])
                rootsT = actp.tile([G_PER_CORE, 2, 128], F32, tag="rootsT",
                                   name=f"rootsT_{b}")
                for j in (0, 1):
                    ps_rt = psH2.tile([G_PER_CORE, 128], MM_DT,
                                      padded_shape=[128, 1024], tag="psH2",
                                      name=f"ps_rt_{b}_{j}")
                    nc.tensor.transpose(ps_rt[:], roots_sb[b][:, j, :],
                                        identity[:])
                    nc.scalar.copy(rootsT[:, j, :], ps_rt[:])
                nc.sync.dma_start(out[:, b * 512 + 256:(b + 1) * 512],
                                  rootsT[:])

    nc.compile()
    return nc


# ----------------------------------------------------------------------------
# Host entry point
# ----------------------------------------------------------------------------

def _prep(inputs, w1_td, b1_td, w2_td, b2_td, w1_bu, b1_bu, w2_bu, b2_bu,
          td_src, td_dst, bu_src, bu_dst, nodes_per_graph):
    n = int(nodes_per_graph)
    X = np.ascontiguousarray(np.asarray(inputs, np.float32))
    N = X.shape[0]
    G = N // n
    assert (n, G, X.shape[1]) == (N_PER_G, N_GRAPHS, IN_FEATS), \
        f"unexpected shapes {X.shape} n={n}"

    adjt_td = build_adjt(td_src, td_dst, n, G)  # [G, s, d] f32
    adjt_bu = build_adjt(bu_src, bu_dst, n, G)
    # [G, 2, s, d] -> per-core [s, g, b, d] flattened
    adjt = np.stack([adjt_td, adjt_bu], axis=1)

    w1p = np.ascontiguousarray(
        np.concatenate([np.asarray(w1_td, np.float32),
                        np.asarray(w1_bu, np.float32)], axis=1))
    w2_td = np.asarray(w2_td, np.float32)
    w2_bu = np.asarray(w2_bu, np.float32)
    w2rp = np.ascontiguousarray(
        np.concatenate([w2_td[H_FEATS:], w2_bu[H_FEATS:]], axis=1))
    # rvec = Xroots @ [W2r_td | W2r_bu] on the host (removes a device prelude)
    rvec = (X[::n].astype(np.float32) @ w2rp).astype(BF16)  # [G, 512]
    biases = [np.asarray(b, np.float32) for b in (b1_td, b2_td, b1_bu, b2_bu)]
    has_bias = any(np.any(b != 0) for b in biases)

    msel = np.zeros((128, G_PER_CORE, G_PER_CORE), np.float32)
    for gi in range(G_PER_CORE):
        msel[:, gi, gi] = 1.0 / N_PER_G
    msel = msel.reshape(128, G_PER_CORE * G_PER_CORE).astype(BF16)

    Xbf = X.astype(BF16)
    in_maps = []
    for c in range(N_CORES):
        gs = slice(c * G_PER_CORE, (c + 1) * G_PER_CORE)
        ns = slice(c * NODES_PER_CORE, (c + 1) * NODES_PER_CORE)
        Xc = Xbf[ns]
        adc = adjt[gs]  # [32, 2, 128, 128]
        m = {
            "xt": np.ascontiguousarray(Xc.T),
            "adjt": np.ascontiguousarray(
                adc.transpose(2, 0, 1, 3).reshape(128, -1)).astype(BF16),
            "w1p": w1p.astype(BF16),
            "w2h_td": np.ascontiguousarray(w2_td[:H_FEATS]).astype(BF16),
            "w2h_bu": np.ascontiguousarray(w2_bu[:H_FEATS]).astype(BF16),
            "rvecp": np.ascontiguousarray(rvec[gs].reshape(1, -1)),
            "meansel": msel,
        }
        if has_bias:
            m["b1c_td"] = np.ascontiguousarray(
                biases[0].reshape(2, 128).T.astype(np.float32))
            m["b1c_bu"] = np.ascontiguousarray(
                biases[2].reshape(2, 128).T.astype(np.float32))
            m["b2b_td"] = np.ascontiguousarray(
                np.broadcast_to(biases[1], (128, H_FEATS)).astype(np.float32))
            m["b2b_bu"] = np.ascontiguousarray(
                np.broadcast_to(biases[3], (128, H_FEATS)).astype(np.float32))
        in_maps.append(m)
    return in_maps, has_bias


_PROGRAM_CACHE = {}


def _get_program(key):
    if key not in _PROGRAM_CACHE:
        _PROGRAM_CACHE[key] = build_program(key)
    return _PROGRAM_CACHE[key]


def kernel(trace=False, tmpdir=None, _return_raw=False, **inputs):
    in_maps, has_bias = _prep(**inputs)
    nc = _get_program(has_bias)
    res = run_bass_kernel_spmd(nc, in_maps, list(range(N_CORES)),
                               trace=trace, tmpdir=tmpdir)
    out = np.concatenate([res.results[i]["out"] for i in range(N_CORES)], axis=0)
    if _return_raw:
        return out, res
    return out
